# revision 19
# baseline (speedup 1.0000x reference)
"""Entity-resolution head on 8 TRN2 NeuronCores.

Pure data-parallel: batch dim (256) is split 32/core; MLP weights are
replicated.  Key optimizations over the bf16 baseline:

- We1/Wp1 (83% of weight bytes) are stored as fp8 e3m4, pre-scaled on the
  host so |w| <= ~14.  Both matrices feed LayerNorm, which is invariant to
  a uniform scale, so no de-scale op is needed anywhere — only the biases
  (added pre-LN) and eps are scaled to match.
- Weights stream in consumption order (We1 first) on the sync HWDGE ring
  while small inputs ride the scalar ring; the We1 matmuls consume weight
  chunks as they arrive.
- Span gathers use fixed 16-row slots: one SWDGE descriptor per batch
  moves the whole span window (rows s..s+15, always in-bounds), so Q7
  descriptor generation drops from ~1000 to ~100 descriptors and no
  memset/bounds-check is needed (garbage rows get 0.0 mask coefficients,
  and gathered bert values are always finite).
- LayerNorm + leaky-relu collapse into one scalar-engine Lrelu activation
  (scale=rstd, bias=-mean*rstd, alpha=0.01); exact gelu is a single Gelu
  activation reading PSUM directly.
- The pron-branch matmuls are scheduled right after the We1 group so the
  PE stays busy (HAM stays at 2.4 GHz) while the vector engine runs the
  ent-branch LayerNorm.
"""

import numpy as np
import ml_dtypes

import concourse.bass as bass
import concourse.mybir as mybir
import concourse.tile as tile
from concourse.bass_utils import run_bass_kernel_spmd
from concourse.masks import make_identity

B, S, H = 256, 512, 1024
HH, LH, NOUT = 512, 512, 3
EPS = 1e-5
NCORES = 8
BC = B // NCORES          # 32 batches per core
SLOT = 16                 # rows gathered per span slot (max span len 15)
NCH = 4                   # 4 partition-chunks of 128 rows (8 slots each)
F32 = mybir.dt.float32
BF16 = mybir.dt.bfloat16
F8 = mybir.dt.float8e3
I32 = mybir.dt.int32

# fp8 e3m4 scales: |W| <= 1/sqrt(fan_in); scaled max ~14 < 15.5 (inf)
S_P1 = 448.0              # Wp1: bound 1/32  -> max 14.0
S_E1 = 1100.0             # We1: bound 1/78.4 -> max ~14.03
EPS_P = EPS * S_P1 * S_P1
EPS_E = EPS * S_E1 * S_E1

# weight matrices in PRELOAD (consumption) order: the pron branch runs
# first (during the gather window), then the big ent stream.
# (name, K, N, k-chunks per DMA, fp8?)
MAT_SPECS = [
    ("Wp1", H, H, 8, True),
    ("Wp2", H, HH, 8, False),
    ("We1", 6 * H, H, 4, True),
    ("We2", H, HH, 8, False),
    ("Wl", 2 * HH, LH, 8, False),
    ("Wc", LH, NOUT, 4, False),
]
# biases / norm params packed into one [8, 1024] f32 tensor
BIAS_ROWS = [("bp1",), ("gp",), ("betap",), ("be1",), ("ge",), ("betae",),
             ("bp2", "be2"), ("bl", "bc")]


def _bcast_rows(ap, p):
    """AP view of DRAM tensor broadcast across p partitions."""
    return bass.AP(tensor=ap.tensor, offset=ap.offset, ap=[[0, p]] + list(ap.ap))


def _build_program(affine=False):
    nc = bass.Bass()

    bert = nc.declare_dram_parameter("bert", [BC, S, H], BF16, isOutput=False)
    idxs = nc.declare_dram_parameter("idxs", [128, 9], I32, isOutput=False)
    masks = nc.declare_dram_parameter("masks", [128, 2, NCH, 3 * BC], BF16,
                                      isOutput=False)
    biases = nc.declare_dram_parameter("biases", [1, 8192], BF16, isOutput=False)
    wd = {}
    for name, K, N, _, is8 in MAT_SPECS:
        wd[name] = nc.declare_dram_parameter(
            f"{name}T", [128, K // 128, N], F8 if is8 else BF16, isOutput=False)
    out = nc.declare_dram_parameter("out", [BC, NOUT], F32, isOutput=True)

    bert2d = bert[:].rearrange("b s h -> (b s) h")   # [16384, H] bf16

    with tile.TileContext(nc) as tc:
        with (
            tc.tile_pool(name="singles", bufs=1) as singles,
            tc.tile_pool(name="acts", bufs=1) as acts,
            tc.tile_pool(name="pA", bufs=1, space="PSUM") as pA,
            tc.tile_pool(name="pB", bufs=1, space="PSUM") as pB,
            tc.tile_pool(name="pshare", bufs=3, space="PSUM") as pshare,
            tc.tile_pool(name="pdummy", bufs=1, space="PSUM") as pdummy,
        ):
            # ---- DMA issuance first (engines are otherwise idle) -------
            # idx rides the SWDGE queue so the gathers (same queue, FIFO)
            # start as soon as it lands instead of waiting on a HWDGE sem.
            idx = singles.tile([128, 9], I32, tag="idx")
            nc.gpsimd.dma_start(idx[:], idxs[:])
            mk = singles.tile([128, 2, NCH, 3 * BC], BF16, tag="mk")
            nc.scalar.dma_start(mk[:], masks[:])
            bias_sb = singles.tile([1, 8192], BF16, tag="bias")
            nc.scalar.dma_start(bias_sb[:], biases[:])

            # ---- constants ---------------------------------------------
            ident32 = singles.tile([32, 32], BF16, tag="ident32")
            make_identity(nc, ident32[:])
            ident96 = singles.tile([96, 96], BF16, tag="ident96")
            make_identity(nc, ident96[:])
            eps_p = singles.tile([BC, 1], F32, tag="eps_p")
            nc.vector.memset(eps_p[:], EPS_P)
            eps_e = singles.tile([BC, 1], F32, tag="eps_e")
            nc.vector.memset(eps_e[:], EPS_E)

            # Walrus on this toolchain allows exactly ONE sync-wait per
            # instruction.  pe_observe() is a throwaway 32x32 transpose that
            # makes the PE observe one fresh semaphore so real matmuls only
            # ever need a single wait.  All observers accumulate into ONE
            # psum tile as a single matmul group so they never create
            # PSUM WAR hazards (which would need a second wait).
            N_OBSERVERS = 4
            dummy_ps = pdummy.tile([32, 32], BF16, tag="dummy")
            obs_count = [0]

            def pe_observe(src_ap, name):
                i = obs_count[0]
                obs_count[0] += 1
                nc.tensor.matmul(
                    dummy_ps[:], lhsT=src_ap, rhs=ident32[:],
                    is_transpose=True,
                    start=(i == 0), stop=(i == N_OBSERVERS - 1),
                    skip_group_check=True)

            pe_observe(ident96[0:32, 0:32], "ident")

            from concourse.tile import add_dep_helper

            def _raw(inst):
                return inst.ins if hasattr(inst, "ins") else inst

            def engine_absorb(eng, *dep_insts):
                deps = [d for d in dep_insts if d is not None]
                if not deps:
                    return None
                dr = None
                for d in deps:
                    dr = eng.drain(fusable=False)
                    add_dep_helper(_raw(dr), _raw(d), sync=True,
                                   reason="engine observes producer")
                return dr

            def order_after(inst, dr):
                if dr is not None and inst is not None:
                    add_dep_helper(_raw(inst), _raw(dr), sync=False,
                                   reason="consumer ordered after absorber")

            ma, mb = mk[:, 0], mk[:, 1]
            BIAS_COL = {"bp1": 0, "be1": 3 * 1024, "bp2": 6 * 1024,
                        "be2": 6 * 1024 + HH, "bl": 7 * 1024,
                        "bc": 7 * 1024 + LH}
            aff_t = None
            if affine:
                aff_t = singles.tile([BC, 4, 1024], BF16, tag="aff")
                for j, r in enumerate((1, 2, 4, 5)):   # gp, betap, ge, betae
                    nc.gpsimd.dma_start(
                        aff_t[:, j:j + 1, :],
                        _bcast_rows(biases[0:1, r * 1024:(r + 1) * 1024], BC))
            ones_row = singles.tile([1, BC], BF16, tag="ones")
            nc.vector.memset(ones_row[:], 1.0)

            # ---- span gathers (SWDGE, 16-row slots) --------------------
            # slot for batch b lives at partitions 16*(b//4).. in chunk b%4;
            # row j of the slot is always s_b + j (in-bounds: s <= 495), so
            # no memset or bounds-check is needed — garbage rows get 0.0
            # mask coefficients and bert values are always finite.
            def gather_span(col0, tag):
                g = singles.tile([128, NCH, H], BF16, tag=tag)
                for c in range(NCH):
                    nc.gpsimd.indirect_dma_start(
                        out=g[:, c, :], out_offset=None,
                        in_=bert2d,
                        in_offset=bass.IndirectOffsetOnAxis(
                            ap=idx[:, col0 + c:col0 + c + 1], axis=0),
                    )
                return g

            GP = singles.tile([BC, H], BF16, tag="gp_rows")
            nc.gpsimd.indirect_dma_start(
                out=GP[:], out_offset=None, in_=bert2d,
                in_offset=bass.IndirectOffsetOnAxis(ap=idx[0:BC, 8:9], axis=0),
            )
            GA = gather_span(0, "ga")
            GB = gather_span(NCH, "gb")

            # ---- weight preload: consumption order, sync HWDGE ---------
            wsb = {}
            wload = {}          # name -> list of (k_start, dma_inst)

            def preload(name):
                _, K, N, kk, is8 = next(s for s in MAT_SPECS if s[0] == name)
                nk = K // 128
                t = singles.tile([128, nk, N], F8 if is8 else BF16,
                                 tag=f"w_{name}")
                wsb[name] = t
                wload[name] = []
                for k0 in range(0, nk, kk):
                    ld = nc.sync.dma_start(t[:, k0:k0 + kk, :],
                                           wd[name][:, k0:k0 + kk, :])
                    wload[name].append((k0, ld))

            for name, _, _, _, _ in MAT_SPECS:
                preload(name)

            def load_for(name, k):
                _, _, _, kk, _ = next(s for s in MAT_SPECS if s[0] == name)
                return wload[name][k // kk][1]

            pe_observe(ma[0:32, 0, 0:32], "ma")
            pe_observe(mb[0:32, 0, 0:32], "mb")

            # ---- span features: S = M.T @ G  -> [96, H] ----------------
            def span_feats(m_tile, g_tiles, tag):
                ps = [pshare.tile([96, 512], F32, tag="share", name=f"ps_{tag}{h}")
                      for h in range(2)]
                for c in range(NCH):
                    for h in range(2):
                        nc.tensor.matmul(
                            ps[h][:],
                            lhsT=m_tile[:, c, :],
                            rhs=g_tiles[:, c, h * 512:(h + 1) * 512],
                            start=(c == 0), stop=(c == NCH - 1),
                        )
                sb = singles.tile([96, H], BF16, tag="sf")
                for h in range(2):
                    nc.vector.tensor_copy(sb[:, h * 512:(h + 1) * 512], ps[h][:])
                return sb

            # transpose span feats -> [128, 8, 96] per side (bf16 in/out)
            def transpose_feats(src, tag):
                dst = singles.tile([128, 8, 96], BF16, tag=f"t_{tag}")
                cp = None
                for h in range(8):
                    pt = pshare.tile([128, 96], BF16, tag="share", name="pt96")
                    nc.tensor.transpose(
                        pt[:], src[:, h * 128:(h + 1) * 128], ident96[:])
                    cp = nc.vector.tensor_copy(dst[:, h, :], pt[:])
                return dst, cp

            # pron rows transposed -> [128, 8, 32]
            pe_observe(GP[0:32, 0:32], "gp_lane")
            PT = singles.tile([128, 8, BC], BF16, tag="ptron")
            PT_cp = None
            for h in range(8):
                pt = pshare.tile([128, 96], BF16, tag="share", name="pt32")
                pt = pt[:, :BC]
                nc.tensor.transpose(
                    pt[:], GP[:, h * 128:(h + 1) * 128], ident32[:])
                PT_cp = nc.vector.tensor_copy(PT[:, h, :], pt[:])

            # transpose a batch-major bf16 [BC, n*128] activation -> [128, n, BC]
            def transpose_into(dst, src, h0, n):
                cp = None
                for h in range(n):
                    pt = pshare.tile([128, 96], BF16, tag="share", name="pt32")
                    pt = pt[:, :BC]
                    nc.tensor.transpose(
                        pt[:], src[:, h * 128:(h + 1) * 128], ident32[:])
                    cp = nc.vector.tensor_copy(dst[:, h0 + h, :], pt[:])
                return cp

            def transpose_act(src, n, tag):
                dst = acts.tile([128, n, BC], BF16, tag=f"tact_{tag}")
                cp = transpose_into(dst, src, 0, n)
                return dst, cp

            # matmul over preloaded weights: lhsT chunks [128, m] bf16
            last_mm = [None]

            def sb_matmul(psum_ap, lhsT_chunks, name, ktiles, n_out,
                          lhsT_deps=(), bias=None):
                for k in range(ktiles):
                    dr_e = None
                    if k == 0:
                        dr_e = engine_absorb(nc.tensor, *lhsT_deps,
                                             last_mm[0])
                    mm = None
                    for h in range(0, n_out, 512):
                        hi = min(h + 512, n_out)
                        mm = nc.tensor.matmul(
                            psum_ap[:, h:hi],
                            lhsT=lhsT_chunks(k),
                            rhs=wsb[name][:, k, h:hi],
                            start=(k == 0),
                            stop=(k == ktiles - 1 and bias is None),
                        )
                        order_after(mm, dr_e)
                    last_mm[0] = mm
                if bias is not None:
                    c0 = BIAS_COL[bias]
                    for h in range(0, n_out, 512):
                        hi = min(h + 512, n_out)
                        mm = nc.tensor.matmul(
                            psum_ap[:, h:hi],
                            lhsT=ones_row[:],
                            rhs=bias_sb[0:1, c0 + h:c0 + hi],
                            start=False, stop=True,
                        )
                    last_mm[0] = mm

            # LayerNorm + leaky-relu -> bf16.  Inputs are uniformly scaled
            # by s (fp8 weight scale); LN is scale-invariant given
            # eps' = eps*s^2, so no de-scale op is needed.  leaky(y) =
            # 0.505*y + 0.495*|y| with y = (x-m)*rstd; the Abs runs on the
            # scalar engine with the normalize fused into its scale/bias
            # APs (Abs lives in every ACT table -> no table thrash).
            def ln_leaky(psum_t, g_idx, beta_idx, n, eps_t, tag):
                nsub = n // 512
                stats = acts.tile([BC, nsub, 6], F32, tag="ln_st")
                xv = psum_t[:].rearrange("p (s f) -> p s f", f=512)
                for s in range(nsub):
                    nc.vector.bn_stats(out=stats[:, s, :], in_=xv[:, s, :])
                mv = acts.tile([BC, 2], F32, tag="ln_mv")
                nc.vector.bn_aggr(out=mv[:], in_=stats[:])
                std = acts.tile([BC, 1], F32, tag="ln_sd")
                nc.scalar.activation(
                    out=std[:], in_=mv[:, 1:2],
                    func=mybir.ActivationFunctionType.Sqrt,
                    bias=eps_t[:], scale=1.0)
                rstd = acts.tile([BC, 1], F32, tag="ln_rs")
                nc.vector.reciprocal(out=rstd[:], in_=std[:])
                xb = acts.tile([BC, n], BF16, tag=f"ln_out_{tag}")
                if affine:
                    x = acts.tile([BC, n], F32, tag="ln_x")
                    nc.vector.tensor_scalar(
                        out=x[:], in0=psum_t[:], scalar1=mv[:, 0:1],
                        scalar2=rstd[:],
                        op0=mybir.AluOpType.subtract, op1=mybir.AluOpType.mult)
                    nc.vector.tensor_mul(x[:], x[:], aff_t[:, g_idx, :n])
                    nc.vector.tensor_add(x[:], x[:], aff_t[:, beta_idx, :n])
                    ab = acts.tile([BC, n], F32, tag="ln_abs")
                    nc.scalar.activation(
                        out=ab[:], in_=x[:],
                        func=mybir.ActivationFunctionType.Abs,
                        bias=0.0, scale=0.495)
                    nc.vector.scalar_tensor_tensor(
                        out=xb[:], in0=x[:], scalar=0.505, in1=ab[:],
                        op0=mybir.AluOpType.mult, op1=mybir.AluOpType.add)
                else:
                    nmr = acts.tile([BC, 1], F32, tag=f"ln_nm_{tag}")
                    nc.vector.tensor_scalar(
                        out=nmr[:], in0=mv[:, 0:1], scalar1=rstd[:],
                        scalar2=-1.0,
                        op0=mybir.AluOpType.mult, op1=mybir.AluOpType.mult)
                    r495 = acts.tile([BC, 2], F32, tag=f"ln_49_{tag}")
                    nc.vector.tensor_scalar(
                        out=r495[:, 0:1], in0=rstd[:], scalar1=0.495,
                        scalar2=None, op0=mybir.AluOpType.mult)
                    nc.vector.tensor_scalar(
                        out=r495[:, 1:2], in0=nmr[:], scalar1=0.495,
                        scalar2=None, op0=mybir.AluOpType.mult)
                    # y = (x - m) * rstd on DVE; ab = 0.495*|y| on ACT
                    y = acts.tile([BC, n], F32, tag="ln_y")
                    nc.vector.tensor_scalar(
                        out=y[:], in0=psum_t[:], scalar1=rstd[:],
                        scalar2=nmr[:],
                        op0=mybir.AluOpType.mult, op1=mybir.AluOpType.add)
                    ab = acts.tile([BC, n], F32, tag="ln_abs")
                    nc.scalar.activation(
                        out=ab[:], in_=psum_t[:],
                        func=mybir.ActivationFunctionType.Abs,
                        bias=r495[:, 1:2], scale=r495[:, 0:1])
                    nc.vector.scalar_tensor_tensor(
                        out=xb[:], in0=y[:], scalar=0.505, in1=ab[:],
                        op0=mybir.AluOpType.mult, op1=mybir.AluOpType.add)
                return xb

            # ---- pron branch (runs during the gather/span window) ------
            ps1p = pA.tile([BC, H], F32, tag="psA", name="ps1p")
            sb_matmul(ps1p, lambda k: PT[:, k, :], "Wp1", 8, H,
                      lhsT_deps=(PT_cp,), bias="bp1")
            X1p = ln_leaky(ps1p, 0, 1, H, eps_p, "p")
            X1pT, X1pT_cp = transpose_act(X1p, 8, "x1p")
            ps2p = pA.tile([BC, HH], F32, tag="psA", name="ps2p")
            sb_matmul(ps2p, lambda k: X1pT[:, k, :], "Wp2", 8, HH,
                      lhsT_deps=(X1pT_cp,), bias="bp2")
            XC = acts.tile([BC, 2 * HH], BF16, tag="xc")
            nc.vector.tensor_copy(XC[:, 0:HH], ps2p[:])
            XCT = acts.tile([128, 8, BC], BF16, tag="tact_xc")
            transpose_into(XCT, XC[:, 0:HH], 0, 4)

            # ---- span features + ent layer 1 (critical path) -----------
            SA = span_feats(ma, GA, "a")
            AT, AT_cp = transpose_feats(SA, "a")
            SB = span_feats(mb, GB, "b")
            BT, BT_cp = transpose_feats(SB, "b")

            def ent_chunk(k):
                blk, h = divmod(k, 8)
                side = AT if blk < 3 else BT
                b = blk % 3
                return side[:, h, b * 32:(b + 1) * 32]

            ps1e = pB.tile([BC, H], F32, tag="psB", name="ps1e")
            sb_matmul(ps1e, ent_chunk, "We1", 48, H,
                      lhsT_deps=(AT_cp, BT_cp), bias="be1")

            # keep-warm spam: covers the ln_e gap so HAM stays at 2.4GHz
            N_SPAM = 48
            for i_s in range(N_SPAM):
                nc.tensor.matmul(
                    dummy_ps[:], lhsT=ident32[:], rhs=ident32[:],
                    is_transpose=True, start=(i_s == 0),
                    stop=(i_s == N_SPAM - 1), skip_group_check=True)

            # ---- ent LN + layer 2 --------------------------------------
            X1e = ln_leaky(ps1e, 2, 3, H, eps_e, "e")
            X1eT, X1eT_cp = transpose_act(X1e, 8, "x1e")
            ps2e = pB.tile([BC, HH], F32, tag="psB", name="ps2e")
            sb_matmul(ps2e, lambda k: X1eT[:, k, :], "We2", 8, HH,
                      lhsT_deps=(X1eT_cp,), bias="be2")
            nc.vector.tensor_copy(XC[:, HH:], ps2e[:])
            XCT_cp = transpose_into(XCT, XC[:, HH:], 4, 4)

            # ---- final hidden + exact gelu (one ACT op) ----------------
            ps3 = pshare.tile([BC, LH], F32, tag="share", name="ps3")
            sb_matmul(ps3, lambda k: XCT[:, k, :], "Wl", 8, LH,
                      lhsT_deps=(XCT_cp,), bias="bl")
            geb = acts.tile([BC, LH], BF16, tag="geb")
            nc.scalar.activation(
                out=geb[:], in_=ps3[:],
                func=mybir.ActivationFunctionType.Gelu,
                bias=0.0, scale=1.0)

            GT, GT_cp = transpose_act(geb, 4, "gt")

            # ---- logits -------------------------------------------------
            ps4 = pshare.tile([BC, NOUT], F32, tag="share", name="ps4")
            dr_wc = engine_absorb(nc.tensor, GT_cp, load_for("Wc", 3),
                                  last_mm[0])
            for k in range(4):
                mm = nc.tensor.matmul(
                    ps4[:], lhsT=GT[:, k, :], rhs=wsb["Wc"][:, k, :],
                    start=(k == 0), stop=False)
                order_after(mm, dr_wc)
            c_bc = BIAS_COL["bc"]
            nc.tensor.matmul(
                ps4[:], lhsT=ones_row[:],
                rhs=bias_sb[0:1, c_bc:c_bc + NOUT],
                start=False, stop=True)
            res = acts.tile([BC, NOUT], F32, tag="res")
            res_add = nc.vector.tensor_copy(res[:], ps4[:])
            engine_absorb(nc.sync, res_add)
            nc.sync.dma_start(out[:], res[:])

    import os
    if not os.environ.get('SKIP_PRUNE'):
        _prune_covered_waits(nc)
    nc.finalize()
    return nc


def _prune_covered_waits(nc):
    """Walrus on this toolchain accepts only one sync-wait on most
    instructions (Drain accepts many).  Within a basic block, same-engine
    instructions execute in order, so a wait already issued by an earlier
    same-engine instruction (e.g. an absorber drain) is redundant on a
    later one and can be dropped."""
    for fn in nc.m.functions:
        for blk in fn.blocks:
            insert = []
            for pos, inst in enumerate(blk.instructions):
                si = inst.sync_info
                if (inst.opcode == "Drain" and si and si.on_wait
                        and len(si.on_wait) > 1):
                    extra = list(si.on_wait[:-1])
                    si.on_wait = [si.on_wait[-1]]
                    insert.append((pos, inst, extra))
            for pos, inst, extra in reversed(insert):
                new_insts = []
                for w in extra:
                    d = mybir.InstDrain(
                        name=nc.get_next_instruction_name(),
                        ins=[], outs=[], bass_is_fusable=False)
                    d.engine = inst.engine
                    d.sync_info = mybir.SyncInfo(on_wait=[w], on_update=[])
                    nc.register_instruction(d)
                    new_insts.append(d)
                blk.instructions[pos:pos] = new_insts

    PRUNABLE = ("DMAHW", "DMASW", "PE_", "DVE_", "Pool_", "Activation_",
                "SP_")

    def prunable(w):
        return (getattr(w, "wait_mode", None) == "sem-ge-imm"
                and w.ant_name.startswith(PRUNABLE))

    for fn in nc.m.functions:
        for blk in fn.blocks:
            observed = {}

            for inst in blk.instructions:
                si = inst.sync_info
                eng = str(inst.engine)
                if si and si.on_wait:
                    kept = []
                    for w in si.on_wait:
                        if prunable(w) and (
                                observed.get((eng, w.ant_name), -1)
                                >= w.wait_value):
                            continue
                        kept.append(w)
                    for w in si.on_wait:
                        key = (eng, w.ant_name)
                        if prunable(w):
                            if observed.get(key, -1) < w.wait_value:
                                observed[key] = w.wait_value
                    if len(kept) != len(si.on_wait):
                        si.on_wait = kept

    # Any non-Drain instruction still holding >1 wait: move all but the last
    # into a chain of single-wait Drains inserted just before it (same
    # engine, so ordering is preserved and walrus sees one wait everywhere).
    for fn in nc.m.functions:
        for blk in fn.blocks:
            insert = []
            for pos, inst in enumerate(blk.instructions):
                si = inst.sync_info
                if (inst.opcode != "Drain" and si and si.on_wait
                        and len(si.on_wait) > 1):
                    extra = list(si.on_wait[:-1])
                    si.on_wait = [si.on_wait[-1]]
                    insert.append((pos, inst, extra))
            for pos, inst, extra in reversed(insert):
                new_insts = []
                for w in extra:
                    d = mybir.InstDrain(
                        name=nc.get_next_instruction_name(),
                        ins=[], outs=[], bass_is_fusable=False)
                    d.engine = inst.engine
                    d.sync_info = mybir.SyncInfo(on_wait=[w], on_update=[])
                    nc.register_instruction(d)
                    new_insts.append(d)
                blk.instructions[pos:pos] = new_insts


_PROGRAMS = {}


def _get_program(affine=False):
    if affine not in _PROGRAMS:
        _PROGRAMS[affine] = _build_program(affine=affine)
    return _PROGRAMS[affine]


def make_in_maps(**inputs):
    """Shard full inputs into per-core input maps (host-side prep)."""
    BF = ml_dtypes.bfloat16
    F8NP = ml_dtypes.float8_e3m4
    bert = np.ascontiguousarray(
        np.asarray(inputs["bert_outputs"], dtype=np.float32)).astype(BF)
    offsets = np.asarray(inputs["offsets"], dtype=np.int32)

    wT = {}
    for name, K, N, _, is8 in MAT_SPECS:
        W = np.asarray(inputs[name], dtype=np.float32)
        if is8:
            s = S_P1 if name == "Wp1" else S_E1
            Wc = (W * s).astype(F8NP)
        else:
            Wc = W.astype(BF)
        wT[f"{name}T"] = np.ascontiguousarray(
            Wc.reshape(K // 128, 128, N).transpose(1, 0, 2))

    bias_pack = np.zeros((8, 1024), np.float32)  # cast to BF below
    for r, names in enumerate(BIAS_ROWS):
        col = 0
        for nm in names:
            v = np.asarray(inputs[nm], dtype=np.float32)
            if nm == "bp1":
                v = v * S_P1
            elif nm == "be1":
                v = v * S_E1
            bias_pack[r, col:col + v.shape[0]] = v
            col += v.shape[0]

    in_maps = []
    for c in range(NCORES):
        ob = offsets[c * BC:(c + 1) * BC]
        m = {"bert": bert[c * BC:(c + 1) * BC]}

        # idx[p, side*4+ch] = flat bert row for partition p of chunk ch:
        # batch b = 4*(p//16)+ch, span row j = p%16 -> b*S + s_b + j
        idx_pack = np.zeros((128, 9), np.int32)
        for ch in range(NCH):
            for p in range(128):
                b = 4 * (p // 16) + ch
                j = p % 16
                idx_pack[p, ch] = b * S + ob[b, 0] + j
                idx_pack[p, NCH + ch] = b * S + ob[b, 2] + j
        idx_pack[:BC, 8] = np.arange(BC, dtype=np.int32) * S + ob[:, 4]
        m["idxs"] = idx_pack

        # masks: batch b's span rows live at partitions 16*(b//4)+j,
        # chunk b%4; garbage rows (j >= ln) keep 0.0 coefficients.
        def span_mask(s_col, e_col):
            M = np.zeros((128, NCH, 3 * BC), np.float32)
            for b in range(BC):
                ln = int(ob[b, e_col] - ob[b, s_col])
                p0, ch = 16 * (b // 4), b % 4
                M[p0, ch, b] = 1.0                          # first
                M[p0 + ln - 1, ch, BC + b] = 1.0            # last
                M[p0:p0 + ln, ch, 2 * BC + b] = 1.0 / ln    # mean
            return M

        MA = span_mask(0, 1)
        MB = span_mask(2, 3)
        m["masks"] = np.ascontiguousarray(
            np.stack([MA, MB], axis=1)).astype(BF)
        m["biases"] = bias_pack.astype(BF).reshape(1, 8192)
        m.update(wT)
        in_maps.append(m)
    return in_maps


def _needs_affine(inputs):
    for nm in ("gp", "ge"):
        if not np.allclose(np.asarray(inputs[nm], np.float32), 1.0):
            return True
    for nm in ("betap", "betae"):
        if not np.allclose(np.asarray(inputs[nm], np.float32), 0.0):
            return True
    return False


def run(in_maps, affine=False, **kwargs):
    nc = _get_program(affine=affine)
    return run_bass_kernel_spmd(nc, in_maps, core_ids=list(range(NCORES)), **kwargs)


def kernel(**inputs):
    res = run(make_in_maps(**inputs), affine=_needs_affine(inputs))
    return np.concatenate([res.results[c]["out"] for c in range(NCORES)],
                          axis=0).astype(np.float32)
